# revision 12
# baseline (speedup 1.0000x reference)
"""Unfold/im2col kernel for Trainium2 (Bass/Tile), 8-core data parallel.

Problem: x [4, 64, 224, 224] f32 -> out [4, 576, 49729] f32 where
out[b, (c*3+kh)*3+kw, oh*223+ow] = pad(x,1)[b, c, oh+kh, ow+kw]
(3x3 kernel, pad 1, stride 1, dilation 1, oh=ow=223).

Sharding: 8 cores = (batch 4) x (channel half 2). Each core handles
32 channels -> [288, 49729] independently; outputs concatenate on the
channel axis (channel-major row layout makes halves contiguous).

The input is zero-padded host-side to [32, 226, 226] per core, so the
device kernel is pure DMA. All 32 padded images live in two SBUF tiles
(padded rows 0..127 / 128..225 on partitions, channels side by side in
the free dim), each filled by ONE load DMA. Each (c, kh) pair is then
written out by one 3D-access-pattern DMA per tile half that emits all
three kw-shifted windows at once (rows x kw x 223 pattern). With only
two loads, each engine's first two out-DMAs carry the load-completion
waits while HWDGE sem lanes are still fresh, so no DMA ever needs more
than the single sync wait walrus allows.
"""

from contextlib import ExitStack

import numpy as np

import concourse.bass as bass
import concourse.tile as tile
from concourse import mybir
from concourse.ap import AP
from concourse.bass_utils import run_bass_kernel_spmd

B, C, IH, IW = 4, 64, 224, 224
N_CORES = 8
CPC = C // 2          # channels per core: 32
PH = IH + 2           # padded height/width: 226
OH = IH - 1           # output spatial: 223
OSZ = OH * OH         # 49729
NROW = CPC * 9        # 288 output rows per core
ROWS0 = 128           # padded rows 0..127 in tile0
ROWS1 = PH - ROWS0    # padded rows 128..225 in tile1 (98)
FREE = CPC * PH       # free dim elements per tile: 7232
PIMG = PH * PH        # padded image elements: 51076

_NC_CACHE = {}


def build_nc() -> bass.Bass:
    nc = bass.Bass()
    x = nc.declare_dram_parameter("xp", [CPC, PH, PH], mybir.dt.float32, isOutput=False)
    out = nc.declare_dram_parameter("out", [NROW, OSZ], mybir.dt.float32, isOutput=True)
    xb = x[:, :, :]
    ob = out[:, :]

    with tile.TileContext(nc) as tc:
        with ExitStack() as ctx:
            pool = ctx.enter_context(tc.tile_pool(name="img", bufs=1))
            t0t = pool.tile([ROWS0, FREE], mybir.dt.float32, name="t0", tag="t0")
            t1t = pool.tile([ROWS1, FREE], mybir.dt.float32, name="t1", tag="t1")
            t0 = t0t[:, :]
            t1 = t1t[:, :]

            # Two loads: tile partition p, free col c*226+w  <-  xp[c, p(+128), w]
            src0 = AP(xb.tensor, xb.offset,
                      [[PH, ROWS0], [PIMG, CPC], [1, PH]])
            dst0 = AP(t0.tensor, t0.offset,
                      [[FREE, ROWS0], [PH, CPC], [1, PH]])
            nc.sync.dma_start(out=dst0, in_=src0)
            src1 = AP(xb.tensor, xb.offset + ROWS0 * PH,
                      [[PH, ROWS1], [PIMG, CPC], [1, PH]])
            dst1 = AP(t1.tensor, t1.offset,
                      [[FREE, ROWS1], [PH, CPC], [1, PH]])
            nc.sync.dma_start(out=dst1, in_=src1)

            # Observer DMAs: walrus allows one sync wait per DMA, and every
            # out-DMA past the 8-lane warmup carries a sem-lane-reuse wait,
            # so it must not also need a load-completion wait. These four
            # 4-byte DMAs (2 per HWDGE engine, one per tile) consume the
            # load waits while lanes are fresh; afterwards both engines'
            # clocks have observed both loads.
            for k, (eng, src) in enumerate(
                    [(nc.sync, t0t), (nc.sync, t1t),
                     (nc.scalar, t0t), (nc.scalar, t1t)]):
                s = pool.tile([1, 1], mybir.dt.float32, name=f"obs{k}", tag=f"obs{k}")
                eng.dma_start(out=s[0:1, 0:1], in_=src[0:1, 0:1])
            # Ordering-only fence: keep the observer DMAs ahead of the
            # out-DMA stream in the final schedule.
            tc.no_sync_barrier()

            # Output windows: row (c*9 + kh*3 + kw), col r*223.. holds
            # padded[kh+r, kw .. kw+222]; rows 0..n0-1 come from tile0,
            # rows n0.. from tile1.
            for c in range(CPC):
                for kh in range(3):
                    eng = nc.sync if (c + kh) % 2 == 0 else nc.scalar
                    n0 = ROWS0 - kh
                    n1 = OH - n0
                    srca = AP(t0.tensor, t0.offset + kh * FREE + c * PH,
                              [[FREE, n0], [1, 3], [1, OH]])
                    dsta = AP(ob.tensor, ob.offset + (c * 9 + kh * 3) * OSZ,
                              [[OH, n0], [OSZ, 3], [1, OH]])
                    eng.dma_start(out=dsta, in_=srca)
                    srcb = AP(t1.tensor, t1.offset + c * PH,
                              [[FREE, n1], [1, 3], [1, OH]])
                    dstb = AP(ob.tensor, ob.offset + (c * 9 + kh * 3) * OSZ + n0 * OH,
                              [[OH, n1], [OSZ, 3], [1, OH]])
                    eng.dma_start(out=dstb, in_=srcb)
    return nc


def _split_multi_waits(nc: bass.Bass) -> None:
    """Walrus allows only one sync-wait command per instruction (the
    kernel-tail drain ends up with one per DMA-completion sem lane).
    Hoist all but the last wait onto fresh single-wait NOPs inserted
    just before the instruction on the same engine — semantically
    identical (the engine blocks on each wait in turn)."""
    from bass_rust import SyncInfo

    k = 0
    for fn in nc.m.functions:
        for blk in fn.blocks:
            insts = blk.instructions
            for idx in range(len(insts) - 1, -1, -1):
                inst = insts[idx]
                si = inst.sync_info
                if si is None or len(si.on_wait) <= 1:
                    continue
                waits = list(si.on_wait)
                for w in waits[:-1]:
                    nop = mybir.InstNoOp(name=f"WSPLIT-{k}")
                    k += 1
                    nop.engine = inst.engine
                    nop.sync_info = SyncInfo(on_wait=[w], on_update=[])
                    insts.insert(idx, nop)
                si.on_wait = [waits[-1]]
                inst.sync_info = si


def get_nc() -> bass.Bass:
    if "nc" not in _NC_CACHE:
        nc = build_nc()
        _split_multi_waits(nc)
        _NC_CACHE["nc"] = nc
    return _NC_CACHE["nc"]


def make_in_maps(x: np.ndarray) -> list[dict]:
    x = np.asarray(x, dtype=np.float32)
    xp = np.pad(x, ((0, 0), (0, 0), (1, 1), (1, 1)))
    maps = []
    for core in range(N_CORES):
        b, half = divmod(core, 2)
        maps.append({"xp": np.ascontiguousarray(xp[b, half * CPC:(half + 1) * CPC])})
    return maps


def gather_out(results: list[dict]) -> np.ndarray:
    out = np.empty((B, C * 9, OSZ), dtype=np.float32)
    for core in range(N_CORES):
        b, half = divmod(core, 2)
        out[b, half * NROW:(half + 1) * NROW] = results[core]["out"]
    return out


def kernel(**inputs) -> np.ndarray:
    x = inputs["x"]
    nc = get_nc()
    res = run_bass_kernel_spmd(nc, make_in_maps(x), list(range(N_CORES)))
    return gather_out(res.results)
